# revision 34
# baseline (speedup 1.0000x reference)
"""BivectorRotarySelfAttention TRN2 kernel.

Sharding: 8 cores = 4 batches x 2 head-halves. Each core computes one batch's
attention for 8 heads (2 kv heads) and a partial output projection; host sums
the two head-half partials per batch.

v2 dataflow (features in partitions, seq in free):
  xT     = recombine(dma_transpose(x_hi), dma_transpose(x_lo))    [f32r]
  k/v/q  = W-blocks.T @ xT   (PSUM-accumulated f32r matmuls)
  rope   = PE permutation-matmul + 2 DVE muls + 1 add -> bf16
  kswap  = partition-swap of rope'd k via 2 SBUF->SBUF DMAs
  scores: per 256-col chunk one psum tile [S0|C0|S1|C1] (4 bf16 K=64 matmuls)
          Act copies [S1|C1] -> SBUF bf16; DVE: tp = [S0|C0]*[S1|C1];
          stt raw = tp_lo + c'*tp_hi written straight into the E tile (bf16)
  exp    in-place per stripe on Act (scale=alpha, bias=key-mask), causal
         triangle via GPSIMD affine_select on the diagonal block
  attnv  + ones-rowsum matmuls per 512-col half; DVE rcp + normalize -> bf16
  y[l,:] = sum_h outT_h.T @ Wo_h  (bf16 matmuls, f32 out)
  Next head's q-projection matmuls are interleaved into the scores phase to
  keep PE busy while DVE/Act post-process score chunks.
"""
import sys
if '/opt/trn_rl_repo' not in sys.path:
    sys.path.insert(0, '/opt/trn_rl_repo')

import numpy as np
import ml_dtypes

import concourse.bass as bass
import concourse.mybir as mybir
import concourse.tile as tile
from concourse import bacc
from concourse.bass_utils import run_bass_kernel_spmd

F32 = mybir.dt.float32
F32R = mybir.dt.float32r
BF16 = mybir.dt.bfloat16

B, L, D, H, HKV = 4, 1024, 2048, 16, 4
HD = D // H            # 128
HD2 = HD // 2          # 64
NH = 8                 # heads per core
NKV = 2                # kv heads per core
NB = L // 128          # 8 key blocks
AluOp = mybir.AluOpType
Act = mybir.ActivationFunctionType

_CACHED = {}


def _chunks_for_stripe(mb):
    """Q-column chunks [(qs, qe)] covering [128*mb, 1024) in <=256-col pieces."""
    q0 = 128 * mb
    out = []
    while q0 < L:
        qe = min(L, q0 + 256)
        out.append((q0, qe))
        q0 = qe
    return out


def build_program():
    nc = bacc.Bacc("TRN2", target_bir_lowering=False, debug=False)

    xh = nc.declare_dram_parameter("xh", [L, D], BF16, isOutput=False)
    xl = nc.declare_dram_parameter("xl", [L, D], BF16, isOutput=False)
    wq = nc.declare_dram_parameter("wq", [128, 16, NH * 128], F32R, isOutput=False)
    wk = nc.declare_dram_parameter("wk", [128, 16, NKV * 128], F32R, isOutput=False)
    wv = nc.declare_dram_parameter("wv", [128, 16, NKV * 128], F32R, isOutput=False)
    wo = nc.declare_dram_parameter("wo", [128, NH, D], BF16, isOutput=False)
    cosq = nc.declare_dram_parameter("cosq", [128, NH, L], BF16, isOutput=False)
    sinq = nc.declare_dram_parameter("sinq", [128, NH, L], BF16, isOutput=False)
    cosk = nc.declare_dram_parameter("cosk", [128, NKV, L], BF16, isOutput=False)
    sink = nc.declare_dram_parameter("sink", [128, NKV, L], BF16, isOutput=False)
    maskb = nc.declare_dram_parameter("maskb", [128, NB], F32, isOutput=False)
    cvec = nc.declare_dram_parameter("cvec", [128, NH], F32, isOutput=False)
    alpha = nc.declare_dram_parameter("alpha", [128, NH], F32, isOutput=False)
    pmrot = nc.declare_dram_parameter("pmrot", [128, 128], BF16, isOutput=False)
    onesb = nc.declare_dram_parameter("onesb", [128, 128], BF16, isOutput=False)
    identb = nc.declare_dram_parameter("identb", [128, 128], BF16, isOutput=False)
    y = nc.declare_dram_parameter("y", [L, D], BF16, isOutput=True)

    with tile.TileContext(nc) as tc:
        with tc.tile_pool(name="persist", bufs=1) as pp:
            consts = {}
            for nm, src, dt_ in [("pmrot", pmrot, BF16), ("onesb", onesb, BF16),
                                 ("identb", identb, BF16), ("maskb", maskb, F32),
                                 ("cvec", cvec, F32), ("alpha", alpha, F32)]:
                consts[nm] = pp.tile(list(src.shape), dt_, tag=nm, name=nm)

            def load_consts():
                for nm, src in [("pmrot", pmrot), ("onesb", onesb),
                                ("identb", identb), ("maskb", maskb),
                                ("cvec", cvec), ("alpha", alpha)]:
                    nc.sync.dma_start(consts[nm][:], src[:])

            # head-0 weights/tables live in the persist pool so their DMAs can
            # be sequenced inside the prologue's serial DMA pipe
            wq_h0 = pp.tile([128, 16, 128], F32R, tag="wq0", name="wq0")
            cq0 = pp.tile([128, L], BF16, tag="cq0", name="cq0")
            sq0 = pp.tile([128, L], BF16, tag="sq0", name="sq0")

            xt = [pp.tile([128, L], F32R, tag=f"xt{i}", name=f"xt{i}")
                  for i in range(16)]
            krt = [pp.tile([128, L], BF16, tag=f"krt{g}", name=f"krt{g}")
                   for g in range(NKV)]
            ksw = [pp.tile([128, L], BF16, tag=f"ksw{g}", name=f"ksw{g}")
                   for g in range(NKV)]
            vblk = [pp.tile([128, 128], BF16, tag=f"vb{i}", name=f"vb{i}")
                    for i in range(NKV * NB)]
            outtn = [pp.tile([128, L], BF16, tag=f"ot{h}", name=f"ot{h}")
                     for h in range(NH)]
            wo_t = [pp.tile([128, D], BF16, tag=f"wo{h}", name=f"wo{h}")
                    for h in range(NH)]

            # ---------------- prologue: xT, k proj + rope + swap, v proj + T
            with (
                tc.tile_pool(name="pro", bufs=1) as ppro,
                tc.tile_pool(name="psp", bufs=1, space="PSUM") as psp,
            ):
                # DMA issue order IS priority (one serial pipe): wk first so
                # kproj can start, x transposes next, wv before the v stream
                # needs it, consts/tables only before their first use
                wk_t = ppro.tile([128, 16, NKV * 128], F32R, tag="wk", name="wk_t")
                wv_t = ppro.tile([128, 16, NKV * 128], F32R, tag="wv", name="wv_t")

                def load_xt(ib):
                    th = ppro.tile([128, L], BF16, tag="xh_t", bufs=3, name="th")
                    tl = ppro.tile([128, L], BF16, tag="xl_t", bufs=3, name="tl")
                    nc.sync.dma_start_transpose(th[:], xh[:, ib * 128:(ib + 1) * 128])
                    nc.sync.dma_start_transpose(tl[:], xl[:, ib * 128:(ib + 1) * 128])
                    nc.vector.tensor_add(xt[ib][:], th[:], tl[:])

                nc.sync.dma_start(wk_t[:], wk[:])
                nc.sync.dma_start(wq_h0[:], wq[:, :, 0:128])
                load_xt(0)
                load_xt(1)
                nc.sync.dma_start(wv_t[:], wv[:])

                psk = [psp.tile([128, L], F32, tag=f"pjk{g}", name=f"psk{g}")
                       for g in range(NKV)]
                psv0 = psp.tile([128, L], F32, tag="pjv0", name="psv0")
                psq0 = psp.tile([128, L], F32, tag="pjq", name="psq0")

                def kv_mms(ps_list, w_t, ib):
                    for g in range(NKV):
                        for c in range(2):
                            nc.tensor.matmul(
                                ps_list[g][:, c * 512:(c + 1) * 512],
                                w_t[:, ib, g * 128:(g + 1) * 128],
                                xt[ib][:, c * 512:(c + 1) * 512],
                                start=(ib == 0), stop=(ib == 15))

                tabs = []

                def load_ktabs():
                    for g in range(NKV):
                        csl = ppro.tile([128, L], BF16, tag="ktab", bufs=4,
                                        name=f"csl{g}")
                        snl = ppro.tile([128, L], BF16, tag="ktab", bufs=4,
                                        name=f"snl{g}")
                        nc.sync.dma_start(csl[:], cosk[:, g, :])
                        nc.sync.dma_start(snl[:], sink[:, g, :])
                        tabs.append((csl, snl))

                # three streams interleaved per ib (k both kv-heads, q head 0,
                # v kv-head 0), paced by the x transposes; v kv-head 1 runs
                # after, overlapped with the k-rope and q-rope chains
                for ib in range(16):
                    if ib + 2 <= 15:
                        load_xt(ib + 2)
                    if ib == 8:
                        load_ktabs()
                    if ib == 11:
                        load_consts()
                        nc.sync.dma_start(cq0[:], cosq[:, 0, :])
                        nc.sync.dma_start(sq0[:], sinq[:, 0, :])
                    kv_mms(psk, wk_t, ib)
                    for c in range(2):
                        nc.tensor.matmul(
                            psq0[:, c * 512:(c + 1) * 512],
                            wq_h0[:, ib, :],
                            xt[ib][:, c * 512:(c + 1) * 512],
                            start=(ib == 0), stop=(ib == 15))
                    if ib >= 4:
                        for c in range(2):
                            nc.tensor.matmul(
                                psv0[:, c * 512:(c + 1) * 512],
                                wv_t[:, ib - 4, 0:128],
                                xt[ib - 4][:, c * 512:(c + 1) * 512],
                                start=(ib == 4), stop=(ib == 15))
                for ib in range(12, 16):
                    for c in range(2):
                        nc.tensor.matmul(
                            psv0[:, c * 512:(c + 1) * 512],
                            wv_t[:, ib, 0:128],
                            xt[ib][:, c * 512:(c + 1) * 512],
                            start=False, stop=(ib == 15))

                # head-0 q lands first so its rope chain overlaps the v g=1
                # projection below
                qt0 = pp.tile([128, L], BF16, tag="qt0p", name="qt0")
                qrt0 = pp.tile([128, L], BF16, tag="qrt0p", name="qrt0")
                qc0 = pp.tile([64, L], BF16, tag="qc0p", name="qc0")
                nc.scalar.copy(qt0[:], psq0[:])

                psv1 = psp.tile([128, L], F32, tag="pjq", name="psv1")
                for ib in range(16):
                    for c in range(2):
                        nc.tensor.matmul(
                            psv1[:, c * 512:(c + 1) * 512],
                            wv_t[:, ib, 128:256],
                            xt[ib][:, c * 512:(c + 1) * 512],
                            start=(ib == 0), stop=(ib == 15))

                for g in range(NKV):
                    kt = ppro.tile([128, L], BF16, tag="kt", bufs=2, name="kt")
                    nc.scalar.copy(kt[:], psk[g][:])
                    psr = psp.tile([128, L], F32, tag=f"pjk{g}", name="pskr")
                    for c in range(2):
                        nc.tensor.matmul(psr[:, c * 512:(c + 1) * 512],
                                         consts["pmrot"][:],
                                         kt[:, c * 512:(c + 1) * 512])
                    csl, snl = tabs[g]
                    t1 = ppro.tile([128, L], BF16, tag="rt", bufs=3, name="t1k")
                    t2 = ppro.tile([128, L], BF16, tag="rt", bufs=3, name="t2k")
                    nc.vector.tensor_mul(t1[:], psr[:], snl[:])
                    nc.vector.tensor_mul(t2[:], kt[:], csl[:])
                    nc.vector.tensor_add(krt[g][:], t1[:], t2[:])
                    # partition-swap halves via SBUF->SBUF DMA
                    nc.sync.dma_start(ksw[g][0:64, :], krt[g][64:128, :])
                    nc.sync.dma_start(ksw[g][64:128, :], krt[g][0:64, :])

                # head-0 rope (reuses the g=0 rope psum slot)
                psr0 = psp.tile([128, L], F32, tag="pjk0", name="psr0")
                for c in range(2):
                    nc.tensor.matmul(psr0[:, c * 512:(c + 1) * 512],
                                     consts["pmrot"][:],
                                     qt0[:, c * 512:(c + 1) * 512])
                t1q = ppro.tile([128, L], BF16, tag="rt", bufs=3, name="t1q")
                t2q = ppro.tile([128, L], BF16, tag="rt", bufs=3, name="t2q")
                nc.vector.tensor_mul(t1q[:], psr0[:], sq0[:])
                nc.vector.tensor_mul(t2q[:], qt0[:], cq0[:])
                nc.vector.tensor_add(qrt0[:], t1q[:], t2q[:])
                nc.vector.tensor_scalar_mul(qc0[:], qrt0[0:64, :],
                                            consts["cvec"][0:64, 0:1])

                for g, psv_g in ((0, psv0), (1, psv1)):
                    vt = ppro.tile([128, L], BF16, tag="vt", bufs=1, name="vt")
                    nc.scalar.copy(vt[:], psv_g[:])
                    # transpose blocks land in bf16 windows of the freed
                    # projection psum tile (no extra PSUM banks)
                    pv = psp.tile([128, L], F32, tag=f"pjk{g}", name="pv")
                    pvb = pv[:].bitcast(BF16)
                    for mb in range(NB):
                        nc.tensor.transpose(pvb[:, mb * 128:(mb + 1) * 128],
                                            vt[:, mb * 128:(mb + 1) * 128],
                                            consts["identb"][:])
                        nc.vector.tensor_copy(vblk[g * NB + mb][:],
                                              pvb[:, mb * 128:(mb + 1) * 128])

            # ---------------- head loop
            with (
                tc.tile_pool(name="hl", bufs=1) as ph,
                tc.tile_pool(name="psh", bufs=1, space="PSUM") as psh,
            ):
                def emit_qproj_dma(h):
                    wq_h = ph.tile([128, 16, 128], F32R, tag="wq_h", bufs=2,
                                   name=f"wq{h}")
                    nc.sync.dma_start(wq_h[:], wq[:, :, h * 128:(h + 1) * 128])
                    cq = ph.tile([128, L], BF16, tag="cq", bufs=2, name=f"cq{h}")
                    sq = ph.tile([128, L], BF16, tag="sq", bufs=2, name=f"sq{h}")
                    nc.sync.dma_start(cq[:], cosq[:, h, :])
                    nc.sync.dma_start(sq[:], sinq[:, h, :])
                    return wq_h, cq, sq

                def qproj_mm_thunks(wq_h, psq):
                    out = []
                    for ib in range(16):
                        for c in range(2):
                            def mm(ib=ib, c=c):
                                nc.tensor.matmul(
                                    psq[:, c * 512:(c + 1) * 512],
                                    wq_h[:, ib, :],
                                    xt[ib][:, c * 512:(c + 1) * 512],
                                    start=(ib == 0), stop=(ib == 15))
                            out.append(mm)
                    return out

                def emit_qtcopy(h, psq):
                    qt = ph.tile([128, L], BF16, tag="qt", bufs=2, name=f"qt{h}")
                    nc.scalar.copy(qt[:], psq[:])
                    return qt

                def emit_rope(h, qt, cq, sq):
                    psr = psh.tile([128, L], F32, tag="pjh", bufs=1,
                                   name=f"psr{h}")
                    for c in range(2):
                        nc.tensor.matmul(psr[:, c * 512:(c + 1) * 512],
                                         consts["pmrot"][:],
                                         qt[:, c * 512:(c + 1) * 512])
                    t1 = ph.tile([128, L], BF16, tag="t1", bufs=1, name="t1")
                    t2 = ph.tile([128, L], BF16, tag="t2", bufs=1, name="t2")
                    qrt = ph.tile([128, L], BF16, tag="qrt", bufs=2,
                                  name=f"qrt{h}")
                    qc = ph.tile([64, L], BF16, tag="qc", bufs=2, name=f"qc{h}")
                    nc.vector.tensor_mul(t1[:], psr[:], sq[:])
                    nc.vector.tensor_mul(t2[:], qt[:], cq[:])
                    nc.vector.tensor_add(qrt[:], t1[:], t2[:])
                    # C0-side q half pre-scaled by c' (folds the bivector
                    # coefficient into the matmul so the chunk combine is a
                    # plain bf16 add)
                    nc.vector.tensor_scalar_mul(qc[:], qrt[0:64, :],
                                                consts["cvec"][0:64, h:h + 1])
                    return qrt, qc

                def attnv_thunks(h, et):
                    """PE matmuls + rcp/normalize for head h's attention*V,
                    as thunks for interleaving into the next head's scores."""
                    g = h // 4
                    out = []
                    for c in range(2):
                        mbs = [mb for mb in range(NB) if 128 * mb < 512 * (c + 1)]

                        def open_group(c=c):
                            po = psh.tile([128, 512], F32, tag="po", bufs=1,
                                          name="po")
                            prs = psh.tile([128, 512], F32, tag="prs", bufs=1,
                                           name="prs")
                            return po, prs
                        state = {}
                        for i, mb in enumerate(mbs):
                            os_ = max(512 * c, 128 * mb)
                            oe = 512 * (c + 1)
                            st, sp = (i == 0), (i == len(mbs) - 1)

                            def mm(c=c, mb=mb, os_=os_, oe=oe, st=st, sp=sp,
                                   state=state, open_group=open_group):
                                if st:
                                    state["po"], state["prs"] = open_group()
                                esl = et[mb][:, os_ - 128 * mb: oe - 128 * mb]
                                nc.tensor.matmul(
                                    state["po"][:, os_ - 512 * c: oe - 512 * c],
                                    vblk[g * NB + mb][:], esl,
                                    start=st, stop=sp)
                                nc.tensor.matmul(
                                    state["prs"][:, os_ - 512 * c: oe - 512 * c],
                                    consts["onesb"][:], esl,
                                    start=st, stop=sp)
                            out.append(mm)

                        def post(h=h, c=c, state=state):
                            rcp = ph.tile([128, 512], F32, tag="rcp", bufs=1,
                                          name="rcp")
                            nc.vector.reciprocal_approx_fast(rcp[:],
                                                             state["prs"][:])
                            nc.gpsimd.tensor_mul(
                                outtn[h][:, 512 * c:512 * (c + 1)],
                                state["po"][:], rcp[:])
                        out.append(post)
                    return out

                # head 0's projection + rope were computed in the prologue
                qrt_cur, qc_cur = qrt0, qc0
                av_prev = []

                for h in range(NH):
                    g = h // 4
                    if h + 1 < NH:
                        wq_hn, cqn, sqn = emit_qproj_dma(h + 1)
                        psqn = psh.tile([128, L], F32, tag="pjh", bufs=1,
                                        name=f"psq{h+1}")
                        qp_thunks = qproj_mm_thunks(wq_hn, psqn)
                    else:
                        qp_thunks = []
                    qp_i = 0
                    av_i = 0
                    qt_next = None
                    rope_next = None

                    et = []
                    for mb in range(NB):
                        w = L - 128 * mb
                        et.append(ph.tile([128, w], BF16, tag=f"et{mb}", bufs=2,
                                          name=f"et{h}_{mb}"))

                    chunk_idx = 0
                    for mb in range(NB):
                        kb = slice(mb * 128, (mb + 1) * 128)
                        for (qs, qe) in _chunks_for_stripe(mb):
                            s = qe - qs
                            ps = psh.tile([128, 1024], F32, tag="sc", bufs=2,
                                          name="sc")
                            nc.tensor.matmul(ps[:, 2 * s:3 * s],
                                             krt[g][64:128, kb],
                                             qrt_cur[64:128, qs:qe])
                            nc.tensor.matmul(ps[:, 3 * s:4 * s],
                                             ksw[g][64:128, kb],
                                             qrt_cur[64:128, qs:qe])
                            nc.tensor.matmul(ps[:, 0:s],
                                             krt[g][0:64, kb],
                                             qrt_cur[0:64, qs:qe])
                            nc.tensor.matmul(ps[:, s:2 * s],
                                             ksw[g][0:64, kb],
                                             qc_cur[0:64, qs:qe])
                            bsB = ph.tile([128, 512], BF16, tag="bsB", bufs=2,
                                          name="bsB")
                            nc.scalar.copy(bsB[:, 0:2 * s], ps[:, 2 * s:4 * s])
                            tp = ph.tile([128, 512], BF16, tag="tp", bufs=2,
                                         name="tp")
                            nc.vector.tensor_mul(tp[:, 0:2 * s],
                                                 ps[:, 0:2 * s],
                                                 bsB[:, 0:2 * s])
                            esl = et[mb][:, qs - 128 * mb: qe - 128 * mb]
                            nc.vector.tensor_add(esl, tp[:, 0:s],
                                                 tp[:, s:2 * s])
                            # interleave filler PE work (next head's q
                            # projection + previous head's attnV),
                            # proportionally paced across the 20 chunks
                            qp_tgt = max(0, chunk_idx - 1) * len(qp_thunks) // 16
                            while qp_i < min(qp_tgt, len(qp_thunks)):
                                qp_thunks[qp_i]()
                                qp_i += 1
                            av_tgt = (chunk_idx + 1) * len(av_prev) // 20
                            while av_i < min(av_tgt, len(av_prev)):
                                av_prev[av_i]()
                                av_i += 1
                            if qp_thunks and qp_i >= len(qp_thunks) \
                                    and qt_next is None:
                                qt_next = emit_qtcopy(h + 1, psqn)
                            elif qt_next is not None and rope_next is None:
                                rope_next = emit_rope(h + 1, qt_next, cqn, sqn)
                            chunk_idx += 1
                        # stripe done: exp in place, causal triangle on diagonal
                        nc.scalar.activation(et[mb][:], et[mb][:], Act.Exp,
                                             bias=consts["maskb"][:, mb:mb + 1],
                                             scale=consts["alpha"][:, h:h + 1])
                        nc.gpsimd.affine_select(
                            et[mb][:, 0:128], et[mb][:, 0:128],
                            pattern=[[1, 128]], compare_op=AluOp.is_ge,
                            fill=0.0, base=0, channel_multiplier=-1)

                    while qp_i < len(qp_thunks):
                        qp_thunks[qp_i]()
                        qp_i += 1
                    while av_i < len(av_prev):
                        av_prev[av_i]()
                        av_i += 1
                    if h + 1 < NH and qt_next is None:
                        qt_next = emit_qtcopy(h + 1, psqn)
                    if 3 <= h <= 6:
                        for hb in (2 * (h - 3), 2 * (h - 3) + 1):
                            nc.sync.dma_start(wo_t[hb][:], wo[:, hb, :])

                    av_prev = attnv_thunks(h, et)
                    if h + 1 < NH:
                        if rope_next is None:
                            rope_next = emit_rope(h + 1, qt_next, cqn, sqn)
                        qrt_cur, qc_cur = rope_next
                    else:
                        for t in av_prev:
                            t()
                        av_prev = []

            # ---------------- epilogue: Wo projection
            with (
                tc.tile_pool(name="ep", bufs=1) as pe2,
                tc.tile_pool(name="pse", bufs=1, space="PSUM") as pse,
            ):
                for lb in range(NB):
                    for c in range(2):
                        psy = pse.tile([128, 1024], F32, tag="py", bufs=2,
                                       name="psy")
                        for cc in range(2):
                            for hh in range(NH):
                                nc.tensor.matmul(
                                    psy[:, cc * 512:(cc + 1) * 512],
                                    outtn[hh][:, lb * 128:(lb + 1) * 128],
                                    wo_t[hh][:, c * 1024 + cc * 512:
                                              c * 1024 + (cc + 1) * 512],
                                    start=(hh == 0), stop=(hh == NH - 1))
                        yt = pe2.tile([128, 1024], BF16, tag="yt", bufs=4,
                                      name="yt")
                        if (lb * 2 + c) % 2 == 0:
                            nc.scalar.copy(yt[:], psy[:])
                        else:
                            nc.vector.tensor_copy(yt[:], psy[:])
                        nc.sync.dma_start(
                            y[lb * 128:(lb + 1) * 128,
                              c * 1024:(c + 1) * 1024], yt[:])

    nc.compile()
    return nc


def _host_prep(x, Wq, Wk, Wv, Wo, q_param, log_scale, cos, sin, mask):
    """Build the 8 per-core input maps."""
    x = np.asarray(x, np.float32)
    Wq = np.asarray(Wq, np.float32)
    Wk = np.asarray(Wk, np.float32)
    Wv = np.asarray(Wv, np.float32)
    Wo = np.asarray(Wo, np.float32)
    cos = np.asarray(cos, np.float32)[0]      # [L, H, 64]
    sin = np.asarray(sin, np.float32)[0]
    qp = np.asarray(q_param, np.float32).reshape(H)
    ls = np.asarray(log_scale, np.float32).reshape(H)
    mask = np.asarray(mask)

    p64 = np.arange(128) % 64

    PM = np.zeros((128, 128), np.float32)
    for dp in range(128):
        base, r = (dp // 64) * 64, dp % 64
        if r < 32:
            PM[base + r + 32, dp] = -1.0
        else:
            PM[base + r - 32, dp] = 1.0
    ONES = np.ones((128, 128), ml_dtypes.bfloat16)
    IDENT = np.eye(128, dtype=ml_dtypes.bfloat16)

    in_maps = []
    for core in range(8):
        b, g2 = core // 2, core % 2
        heads = list(range(g2 * NH, (g2 + 1) * NH))
        kvs = list(range(g2 * NKV, (g2 + 1) * NKV))

        xb = x[b]
        xhv = xb.astype(ml_dtypes.bfloat16)
        xlo = (xb - xhv.astype(np.float32)).astype(ml_dtypes.bfloat16)

        wq_c = Wq[:, g2 * NH * 128:(g2 + 1) * NH * 128]
        wk_c = Wk[:, g2 * NKV * 128:(g2 + 1) * NKV * 128]
        wv_c = Wv[:, g2 * NKV * 128:(g2 + 1) * NKV * 128]
        wo_c = Wo[g2 * NH * 128:(g2 + 1) * NH * 128, :]

        wq_p = wq_c.reshape(16, 128, NH * 128).transpose(1, 0, 2).copy()
        wk_p = wk_c.reshape(16, 128, NKV * 128).transpose(1, 0, 2).copy()
        wv_p = wv_c.reshape(16, 128, NKV * 128).transpose(1, 0, 2).copy()
        wo_p = wo_c.reshape(NH, 128, D).transpose(1, 0, 2).astype(ml_dtypes.bfloat16)

        bf = ml_dtypes.bfloat16
        cosq_p = np.ascontiguousarray(
            cos[:, heads, :][:, :, p64].transpose(2, 1, 0)).astype(bf)
        sinq_p = np.ascontiguousarray(
            sin[:, heads, :][:, :, p64].transpose(2, 1, 0)).astype(bf)
        cosk_p = np.ascontiguousarray(
            cos[:, kvs, :][:, :, p64].transpose(2, 1, 0)).astype(bf)
        sink_p = np.ascontiguousarray(
            sin[:, kvs, :][:, :, p64].transpose(2, 1, 0)).astype(bf)

        mb = np.where(mask[b].reshape(NB, 128).T.astype(bool), 0.0, -1e9)
        mb = mb.astype(np.float32)

        cv = np.ones((128, NH), np.float32)
        cv[0:64, :] = (-2.0 * np.tanh(qp[heads]))[None, :]
        alp = np.tile((np.exp(ls[heads]) / HD)[None, :], (128, 1))

        in_maps.append({
            "xh": xhv, "xl": xlo,
            "wq": wq_p.astype(np.float32), "wk": wk_p.astype(np.float32),
            "wv": wv_p.astype(np.float32), "wo": wo_p,
            "cosq": cosq_p, "sinq": sinq_p, "cosk": cosk_p, "sink": sink_p,
            "maskb": mb, "cvec": cv,
            "alpha": alp.astype(np.float32),
            "pmrot": PM.astype(bf), "onesb": ONES, "identb": IDENT,
        })
    return in_maps


def kernel(**inputs):
    if "nc" not in _CACHED:
        _CACHED["nc"] = build_program()
    nc = _CACHED["nc"]
    in_maps = _host_prep(**inputs)
    res = run_bass_kernel_spmd(nc, in_maps, list(range(8))).results
    out = np.empty((B, L, D), np.float32)
    for b in range(B):
        out[b] = (res[2 * b]["y"].astype(np.float32)
                  + res[2 * b + 1]["y"].astype(np.float32))
    return out


# revision 35
# speedup vs baseline: 1.0022x; 1.0022x over previous
"""BivectorRotarySelfAttention TRN2 kernel.

Sharding: 8 cores = 4 batches x 2 head-halves. Each core computes one batch's
attention for 8 heads (2 kv heads) and a partial output projection; host sums
the two head-half partials per batch.

v2 dataflow (features in partitions, seq in free):
  xT     = recombine(dma_transpose(x_hi), dma_transpose(x_lo))    [f32r]
  k/v/q  = W-blocks.T @ xT   (PSUM-accumulated f32r matmuls)
  rope   = PE permutation-matmul + 2 DVE muls + 1 add -> bf16
  kswap  = partition-swap of rope'd k via 2 SBUF->SBUF DMAs
  scores: per 256-col chunk one psum tile [S0|C0|S1|C1] (4 bf16 K=64 matmuls)
          Act copies [S1|C1] -> SBUF bf16; DVE: tp = [S0|C0]*[S1|C1];
          stt raw = tp_lo + c'*tp_hi written straight into the E tile (bf16)
  exp    in-place per stripe on Act (scale=alpha, bias=key-mask), causal
         triangle via GPSIMD affine_select on the diagonal block
  attnv  + ones-rowsum matmuls per 512-col half; DVE rcp + normalize -> bf16
  y[l,:] = sum_h outT_h.T @ Wo_h  (bf16 matmuls, f32 out)
  Next head's q-projection matmuls are interleaved into the scores phase to
  keep PE busy while DVE/Act post-process score chunks.
"""
import sys
if '/opt/trn_rl_repo' not in sys.path:
    sys.path.insert(0, '/opt/trn_rl_repo')

import numpy as np
import ml_dtypes

import concourse.bass as bass
import concourse.mybir as mybir
import concourse.tile as tile
from concourse import bacc
from concourse.bass_utils import run_bass_kernel_spmd

F32 = mybir.dt.float32
F32R = mybir.dt.float32r
BF16 = mybir.dt.bfloat16

B, L, D, H, HKV = 4, 1024, 2048, 16, 4
HD = D // H            # 128
HD2 = HD // 2          # 64
NH = 8                 # heads per core
NKV = 2                # kv heads per core
NB = L // 128          # 8 key blocks
AluOp = mybir.AluOpType
Act = mybir.ActivationFunctionType

_CACHED = {}


def _chunks_for_stripe(mb):
    """Q-column chunks [(qs, qe)] covering [128*mb, 1024) in <=256-col pieces."""
    q0 = 128 * mb
    out = []
    while q0 < L:
        qe = min(L, q0 + 256)
        out.append((q0, qe))
        q0 = qe
    return out


def build_program():
    nc = bacc.Bacc("TRN2", target_bir_lowering=False, debug=False)

    xh = nc.declare_dram_parameter("xh", [L, D], BF16, isOutput=False)
    xl = nc.declare_dram_parameter("xl", [L, D], BF16, isOutput=False)
    wq = nc.declare_dram_parameter("wq", [128, 16, NH * 128], F32R, isOutput=False)
    wk = nc.declare_dram_parameter("wk", [128, 16, NKV * 128], F32R, isOutput=False)
    wv = nc.declare_dram_parameter("wv", [128, 16, NKV * 128], F32R, isOutput=False)
    wo = nc.declare_dram_parameter("wo", [128, NH, D], BF16, isOutput=False)
    cosq = nc.declare_dram_parameter("cosq", [128, NH, L], BF16, isOutput=False)
    sinq = nc.declare_dram_parameter("sinq", [128, NH, L], BF16, isOutput=False)
    cosk = nc.declare_dram_parameter("cosk", [128, NKV, L], BF16, isOutput=False)
    sink = nc.declare_dram_parameter("sink", [128, NKV, L], BF16, isOutput=False)
    maskb = nc.declare_dram_parameter("maskb", [128, NB], F32, isOutput=False)
    cvec = nc.declare_dram_parameter("cvec", [128, NH], F32, isOutput=False)
    alpha = nc.declare_dram_parameter("alpha", [128, NH], F32, isOutput=False)
    pmrot = nc.declare_dram_parameter("pmrot", [128, 128], BF16, isOutput=False)
    onesb = nc.declare_dram_parameter("onesb", [128, 128], BF16, isOutput=False)
    identb = nc.declare_dram_parameter("identb", [128, 128], BF16, isOutput=False)
    y = nc.declare_dram_parameter("y", [L, D], BF16, isOutput=True)

    with tile.TileContext(nc) as tc:
        with tc.tile_pool(name="persist", bufs=1) as pp:
            consts = {}
            for nm, src, dt_ in [("pmrot", pmrot, BF16), ("onesb", onesb, BF16),
                                 ("identb", identb, BF16), ("maskb", maskb, F32),
                                 ("cvec", cvec, F32), ("alpha", alpha, F32)]:
                consts[nm] = pp.tile(list(src.shape), dt_, tag=nm, name=nm)

            def load_consts():
                for nm, src in [("pmrot", pmrot), ("onesb", onesb),
                                ("identb", identb), ("maskb", maskb),
                                ("cvec", cvec), ("alpha", alpha)]:
                    nc.sync.dma_start(consts[nm][:], src[:])

            # head-0 weights/tables live in the persist pool so their DMAs can
            # be sequenced inside the prologue's serial DMA pipe
            wq_h0 = pp.tile([128, 16, 128], F32R, tag="wq0", name="wq0")
            cq0 = pp.tile([128, L], BF16, tag="cq0", name="cq0")
            sq0 = pp.tile([128, L], BF16, tag="sq0", name="sq0")

            xt = [pp.tile([128, L], F32R, tag=f"xt{i}", name=f"xt{i}")
                  for i in range(16)]
            krt = [pp.tile([128, L], BF16, tag=f"krt{g}", name=f"krt{g}")
                   for g in range(NKV)]
            ksw = [pp.tile([128, L], BF16, tag=f"ksw{g}", name=f"ksw{g}")
                   for g in range(NKV)]
            vblk = [pp.tile([128, 128], BF16, tag=f"vb{i}", name=f"vb{i}")
                    for i in range(NKV * NB)]
            outtn = [pp.tile([128, L], BF16, tag=f"ot{h}", name=f"ot{h}")
                     for h in range(NH)]
            wo_t = [pp.tile([128, D], BF16, tag=f"wo{h}", name=f"wo{h}")
                    for h in range(NH)]

            # ---------------- prologue: xT, k proj + rope + swap, v proj + T
            with (
                tc.tile_pool(name="pro", bufs=1) as ppro,
                tc.tile_pool(name="psp", bufs=1, space="PSUM") as psp,
            ):
                # DMA issue order IS priority (one serial pipe): wk first so
                # kproj can start, x transposes next, wv before the v stream
                # needs it, consts/tables only before their first use
                wk_t = ppro.tile([128, 16, NKV * 128], F32R, tag="wk", name="wk_t")
                wv_t = ppro.tile([128, 16, NKV * 128], F32R, tag="wv", name="wv_t")

                def load_xt(ib):
                    th = ppro.tile([128, L], BF16, tag="xh_t", bufs=3, name="th")
                    tl = ppro.tile([128, L], BF16, tag="xl_t", bufs=3, name="tl")
                    nc.sync.dma_start_transpose(th[:], xh[:, ib * 128:(ib + 1) * 128])
                    nc.sync.dma_start_transpose(tl[:], xl[:, ib * 128:(ib + 1) * 128])
                    nc.vector.tensor_add(xt[ib][:], th[:], tl[:])

                nc.sync.dma_start(wk_t[:], wk[:])
                nc.sync.dma_start(wq_h0[:], wq[:, :, 0:128])
                load_xt(0)
                load_xt(1)
                nc.sync.dma_start(wv_t[:], wv[:])

                psk = [psp.tile([128, L], F32, tag=f"pjk{g}", name=f"psk{g}")
                       for g in range(NKV)]
                psv0 = psp.tile([128, L], F32, tag="pjv0", name="psv0")
                psq0 = psp.tile([128, L], F32, tag="pjq", name="psq0")

                def kv_mms(ps_list, w_t, ib):
                    for g in range(NKV):
                        for c in range(2):
                            nc.tensor.matmul(
                                ps_list[g][:, c * 512:(c + 1) * 512],
                                w_t[:, ib, g * 128:(g + 1) * 128],
                                xt[ib][:, c * 512:(c + 1) * 512],
                                start=(ib == 0), stop=(ib == 15))

                tabs = []

                def load_ktabs():
                    for g in range(NKV):
                        csl = ppro.tile([128, L], BF16, tag="ktab", bufs=4,
                                        name=f"csl{g}")
                        snl = ppro.tile([128, L], BF16, tag="ktab", bufs=4,
                                        name=f"snl{g}")
                        nc.sync.dma_start(csl[:], cosk[:, g, :])
                        nc.sync.dma_start(snl[:], sink[:, g, :])
                        tabs.append((csl, snl))

                # three streams interleaved per ib (k both kv-heads, q head 0,
                # v kv-head 0), paced by the x transposes; v kv-head 1 runs
                # after, overlapped with the k-rope and q-rope chains
                for ib in range(16):
                    if ib + 2 <= 15:
                        load_xt(ib + 2)
                    if ib == 8:
                        load_ktabs()
                    if ib == 11:
                        load_consts()
                        nc.sync.dma_start(cq0[:], cosq[:, 0, :])
                        nc.sync.dma_start(sq0[:], sinq[:, 0, :])
                    kv_mms(psk, wk_t, ib)
                    for c in range(2):
                        nc.tensor.matmul(
                            psq0[:, c * 512:(c + 1) * 512],
                            wq_h0[:, ib, :],
                            xt[ib][:, c * 512:(c + 1) * 512],
                            start=(ib == 0), stop=(ib == 15))
                    if ib >= 4:
                        for c in range(2):
                            nc.tensor.matmul(
                                psv0[:, c * 512:(c + 1) * 512],
                                wv_t[:, ib - 4, 0:128],
                                xt[ib - 4][:, c * 512:(c + 1) * 512],
                                start=(ib == 4), stop=(ib == 15))
                for ib in range(12, 16):
                    for c in range(2):
                        nc.tensor.matmul(
                            psv0[:, c * 512:(c + 1) * 512],
                            wv_t[:, ib, 0:128],
                            xt[ib][:, c * 512:(c + 1) * 512],
                            start=False, stop=(ib == 15))

                # head-0 q lands first so its rope chain overlaps the v g=1
                # projection below
                qt0 = pp.tile([128, L], BF16, tag="qt0p", name="qt0")
                qrt0 = pp.tile([128, L], BF16, tag="qrt0p", name="qrt0")
                qc0 = pp.tile([64, L], BF16, tag="qc0p", name="qc0")
                nc.scalar.copy(qt0[:], psq0[:])

                psv1 = psp.tile([128, L], F32, tag="pjq", name="psv1")
                for ib in range(16):
                    for c in range(2):
                        nc.tensor.matmul(
                            psv1[:, c * 512:(c + 1) * 512],
                            wv_t[:, ib, 128:256],
                            xt[ib][:, c * 512:(c + 1) * 512],
                            start=(ib == 0), stop=(ib == 15))

                for g in range(NKV):
                    kt = ppro.tile([128, L], BF16, tag="kt", bufs=2, name="kt")
                    nc.scalar.copy(kt[:], psk[g][:])
                    psr = psp.tile([128, L], F32, tag=f"pjk{g}", name="pskr")
                    for c in range(2):
                        nc.tensor.matmul(psr[:, c * 512:(c + 1) * 512],
                                         consts["pmrot"][:],
                                         kt[:, c * 512:(c + 1) * 512])
                    csl, snl = tabs[g]
                    t1 = ppro.tile([128, L], BF16, tag="rt", bufs=3, name="t1k")
                    t2 = ppro.tile([128, L], BF16, tag="rt", bufs=3, name="t2k")
                    nc.vector.tensor_mul(t1[:], psr[:], snl[:])
                    nc.vector.tensor_mul(t2[:], kt[:], csl[:])
                    nc.vector.tensor_add(krt[g][:], t1[:], t2[:])
                    # partition-swap halves via SBUF->SBUF DMA
                    nc.sync.dma_start(ksw[g][0:64, :], krt[g][64:128, :])
                    nc.sync.dma_start(ksw[g][64:128, :], krt[g][0:64, :])

                # head-0 rope (reuses the g=0 rope psum slot)
                psr0 = psp.tile([128, L], F32, tag="pjk0", name="psr0")
                for c in range(2):
                    nc.tensor.matmul(psr0[:, c * 512:(c + 1) * 512],
                                     consts["pmrot"][:],
                                     qt0[:, c * 512:(c + 1) * 512])
                t1q = ppro.tile([128, L], BF16, tag="rt", bufs=3, name="t1q")
                t2q = ppro.tile([128, L], BF16, tag="rt", bufs=3, name="t2q")
                nc.vector.tensor_mul(t1q[:], psr0[:], sq0[:])
                nc.vector.tensor_mul(t2q[:], qt0[:], cq0[:])
                nc.vector.tensor_add(qrt0[:], t1q[:], t2q[:])
                nc.vector.tensor_scalar_mul(qc0[:], qrt0[0:64, :],
                                            consts["cvec"][0:64, 0:1])

                for g, psv_g in ((0, psv0), (1, psv1)):
                    vt = ppro.tile([128, L], BF16, tag="vt", bufs=1, name="vt")
                    nc.scalar.copy(vt[:], psv_g[:])
                    # transpose blocks land in bf16 windows of the freed
                    # projection psum tile (no extra PSUM banks)
                    pv = psp.tile([128, L], F32, tag=f"pjk{g}", name="pv")
                    pvb = pv[:].bitcast(BF16)
                    for mb in range(NB):
                        nc.tensor.transpose(pvb[:, mb * 128:(mb + 1) * 128],
                                            vt[:, mb * 128:(mb + 1) * 128],
                                            consts["identb"][:])
                        nc.vector.tensor_copy(vblk[g * NB + mb][:],
                                              pvb[:, mb * 128:(mb + 1) * 128])

            # ---------------- head loop
            with (
                tc.tile_pool(name="hl", bufs=1) as ph,
                tc.tile_pool(name="psh", bufs=1, space="PSUM") as psh,
            ):
                def emit_qproj_dma(h):
                    wq_h = ph.tile([128, 16, 128], F32R, tag="wq_h", bufs=2,
                                   name=f"wq{h}")
                    nc.sync.dma_start(wq_h[:], wq[:, :, h * 128:(h + 1) * 128])
                    cq = ph.tile([128, L], BF16, tag="cq", bufs=2, name=f"cq{h}")
                    sq = ph.tile([128, L], BF16, tag="sq", bufs=2, name=f"sq{h}")
                    nc.sync.dma_start(cq[:], cosq[:, h, :])
                    nc.sync.dma_start(sq[:], sinq[:, h, :])
                    return wq_h, cq, sq

                def qproj_mm_thunks(wq_h, psq):
                    out = []
                    for ib in range(16):
                        for c in range(2):
                            def mm(ib=ib, c=c):
                                nc.tensor.matmul(
                                    psq[:, c * 512:(c + 1) * 512],
                                    wq_h[:, ib, :],
                                    xt[ib][:, c * 512:(c + 1) * 512],
                                    start=(ib == 0), stop=(ib == 15))
                            out.append(mm)
                    return out

                def emit_qtcopy(h, psq):
                    qt = ph.tile([128, L], BF16, tag="qt", bufs=2, name=f"qt{h}")
                    nc.scalar.copy(qt[:], psq[:])
                    return qt

                def emit_rope(h, qt, cq, sq):
                    psr = psh.tile([128, L], F32, tag="pjh", bufs=1,
                                   name=f"psr{h}")
                    for c in range(2):
                        nc.tensor.matmul(psr[:, c * 512:(c + 1) * 512],
                                         consts["pmrot"][:],
                                         qt[:, c * 512:(c + 1) * 512])
                    t1 = ph.tile([128, L], BF16, tag="t1", bufs=1, name="t1")
                    t2 = ph.tile([128, L], BF16, tag="t2", bufs=1, name="t2")
                    qrt = ph.tile([128, L], BF16, tag="qrt", bufs=2,
                                  name=f"qrt{h}")
                    qc = ph.tile([64, L], BF16, tag="qc", bufs=1, name=f"qc{h}")
                    nc.vector.tensor_mul(t1[:], psr[:], sq[:])
                    nc.vector.tensor_mul(t2[:], qt[:], cq[:])
                    nc.vector.tensor_add(qrt[:], t1[:], t2[:])
                    # C0-side q half pre-scaled by c' (folds the bivector
                    # coefficient into the matmul so the chunk combine is a
                    # plain bf16 add)
                    nc.vector.tensor_scalar_mul(qc[:], qrt[0:64, :],
                                                consts["cvec"][0:64, h:h + 1])
                    return qrt, qc

                def attnv_thunks(h, et):
                    """PE matmuls + rcp/normalize for head h's attention*V,
                    as thunks for interleaving into the next head's scores."""
                    g = h // 4
                    out = []
                    for c in range(2):
                        mbs = [mb for mb in range(NB) if 128 * mb < 512 * (c + 1)]

                        def open_group(c=c):
                            po = psh.tile([128, 512], F32, tag="po", bufs=1,
                                          name="po")
                            prs = psh.tile([128, 512], F32, tag="prs", bufs=1,
                                           name="prs")
                            return po, prs
                        state = {}
                        for i, mb in enumerate(mbs):
                            os_ = max(512 * c, 128 * mb)
                            oe = 512 * (c + 1)
                            st, sp = (i == 0), (i == len(mbs) - 1)

                            def mm(c=c, mb=mb, os_=os_, oe=oe, st=st, sp=sp,
                                   state=state, open_group=open_group):
                                if st:
                                    state["po"], state["prs"] = open_group()
                                esl = et[mb][:, os_ - 128 * mb: oe - 128 * mb]
                                nc.tensor.matmul(
                                    state["po"][:, os_ - 512 * c: oe - 512 * c],
                                    vblk[g * NB + mb][:], esl,
                                    start=st, stop=sp)
                                nc.tensor.matmul(
                                    state["prs"][:, os_ - 512 * c: oe - 512 * c],
                                    consts["onesb"][:], esl,
                                    start=st, stop=sp)
                            out.append(mm)

                        def post(h=h, c=c, state=state):
                            rcp = ph.tile([128, 512], F32, tag="rcp", bufs=1,
                                          name="rcp")
                            nc.vector.reciprocal_approx_fast(rcp[:],
                                                             state["prs"][:])
                            nc.gpsimd.tensor_mul(
                                outtn[h][:, 512 * c:512 * (c + 1)],
                                state["po"][:], rcp[:])
                        out.append(post)
                    return out

                # head 0's projection + rope were computed in the prologue
                qrt_cur, qc_cur = qrt0, qc0
                av_prev = []

                for h in range(NH):
                    g = h // 4
                    if h + 1 < NH:
                        wq_hn, cqn, sqn = emit_qproj_dma(h + 1)
                        psqn = psh.tile([128, L], F32, tag="pjh", bufs=1,
                                        name=f"psq{h+1}")
                        qp_thunks = qproj_mm_thunks(wq_hn, psqn)
                    else:
                        qp_thunks = []
                    qp_i = 0
                    av_i = 0
                    qt_next = None
                    rope_next = None

                    et = []
                    for mb in range(NB):
                        w = L - 128 * mb
                        et.append(ph.tile([128, w], BF16, tag=f"et{mb}", bufs=2,
                                          name=f"et{h}_{mb}"))

                    chunk_idx = 0
                    for mb in range(NB):
                        kb = slice(mb * 128, (mb + 1) * 128)
                        for (qs, qe) in _chunks_for_stripe(mb):
                            s = qe - qs
                            ps = psh.tile([128, 1024], F32, tag="sc", bufs=2,
                                          name="sc")
                            nc.tensor.matmul(ps[:, 2 * s:3 * s],
                                             krt[g][64:128, kb],
                                             qrt_cur[64:128, qs:qe])
                            nc.tensor.matmul(ps[:, 3 * s:4 * s],
                                             ksw[g][64:128, kb],
                                             qrt_cur[64:128, qs:qe])
                            nc.tensor.matmul(ps[:, 0:s],
                                             krt[g][0:64, kb],
                                             qrt_cur[0:64, qs:qe])
                            nc.tensor.matmul(ps[:, s:2 * s],
                                             ksw[g][0:64, kb],
                                             qc_cur[0:64, qs:qe])
                            bsB = ph.tile([128, 512], BF16, tag="bsB", bufs=3,
                                          name="bsB")
                            nc.scalar.copy(bsB[:, 0:2 * s], ps[:, 2 * s:4 * s])
                            tp = ph.tile([128, 512], BF16, tag="tp", bufs=3,
                                         name="tp")
                            nc.vector.tensor_mul(tp[:, 0:2 * s],
                                                 ps[:, 0:2 * s],
                                                 bsB[:, 0:2 * s])
                            esl = et[mb][:, qs - 128 * mb: qe - 128 * mb]
                            nc.vector.tensor_add(esl, tp[:, 0:s],
                                                 tp[:, s:2 * s])
                            # interleave filler PE work (next head's q
                            # projection + previous head's attnV),
                            # proportionally paced across the 20 chunks
                            qp_tgt = max(0, chunk_idx - 1) * len(qp_thunks) // 16
                            while qp_i < min(qp_tgt, len(qp_thunks)):
                                qp_thunks[qp_i]()
                                qp_i += 1
                            av_tgt = (chunk_idx + 1) * len(av_prev) // 20
                            while av_i < min(av_tgt, len(av_prev)):
                                av_prev[av_i]()
                                av_i += 1
                            if qp_thunks and qp_i >= len(qp_thunks) \
                                    and qt_next is None:
                                qt_next = emit_qtcopy(h + 1, psqn)
                            elif qt_next is not None and rope_next is None:
                                rope_next = emit_rope(h + 1, qt_next, cqn, sqn)
                            chunk_idx += 1
                        # stripe done: exp in place, causal triangle on diagonal
                        nc.scalar.activation(et[mb][:], et[mb][:], Act.Exp,
                                             bias=consts["maskb"][:, mb:mb + 1],
                                             scale=consts["alpha"][:, h:h + 1])
                        nc.gpsimd.affine_select(
                            et[mb][:, 0:128], et[mb][:, 0:128],
                            pattern=[[1, 128]], compare_op=AluOp.is_ge,
                            fill=0.0, base=0, channel_multiplier=-1)

                    while qp_i < len(qp_thunks):
                        qp_thunks[qp_i]()
                        qp_i += 1
                    while av_i < len(av_prev):
                        av_prev[av_i]()
                        av_i += 1
                    if h + 1 < NH and qt_next is None:
                        qt_next = emit_qtcopy(h + 1, psqn)
                    if 3 <= h <= 6:
                        for hb in (2 * (h - 3), 2 * (h - 3) + 1):
                            nc.sync.dma_start(wo_t[hb][:], wo[:, hb, :])

                    av_prev = attnv_thunks(h, et)
                    if h + 1 < NH:
                        if rope_next is None:
                            rope_next = emit_rope(h + 1, qt_next, cqn, sqn)
                        qrt_cur, qc_cur = rope_next
                    else:
                        for t in av_prev:
                            t()
                        av_prev = []

            # ---------------- epilogue: Wo projection
            with (
                tc.tile_pool(name="ep", bufs=1) as pe2,
                tc.tile_pool(name="pse", bufs=1, space="PSUM") as pse,
            ):
                for lb in range(NB):
                    for c in range(2):
                        psy = pse.tile([128, 1024], F32, tag="py", bufs=2,
                                       name="psy")
                        for cc in range(2):
                            for hh in range(NH):
                                nc.tensor.matmul(
                                    psy[:, cc * 512:(cc + 1) * 512],
                                    outtn[hh][:, lb * 128:(lb + 1) * 128],
                                    wo_t[hh][:, c * 1024 + cc * 512:
                                              c * 1024 + (cc + 1) * 512],
                                    start=(hh == 0), stop=(hh == NH - 1))
                        yt = pe2.tile([128, 1024], BF16, tag="yt", bufs=4,
                                      name="yt")
                        if (lb * 2 + c) % 2 == 0:
                            nc.scalar.copy(yt[:], psy[:])
                        else:
                            nc.vector.tensor_copy(yt[:], psy[:])
                        nc.sync.dma_start(
                            y[lb * 128:(lb + 1) * 128,
                              c * 1024:(c + 1) * 1024], yt[:])

    nc.compile()
    return nc


def _host_prep(x, Wq, Wk, Wv, Wo, q_param, log_scale, cos, sin, mask):
    """Build the 8 per-core input maps."""
    x = np.asarray(x, np.float32)
    Wq = np.asarray(Wq, np.float32)
    Wk = np.asarray(Wk, np.float32)
    Wv = np.asarray(Wv, np.float32)
    Wo = np.asarray(Wo, np.float32)
    cos = np.asarray(cos, np.float32)[0]      # [L, H, 64]
    sin = np.asarray(sin, np.float32)[0]
    qp = np.asarray(q_param, np.float32).reshape(H)
    ls = np.asarray(log_scale, np.float32).reshape(H)
    mask = np.asarray(mask)

    p64 = np.arange(128) % 64

    PM = np.zeros((128, 128), np.float32)
    for dp in range(128):
        base, r = (dp // 64) * 64, dp % 64
        if r < 32:
            PM[base + r + 32, dp] = -1.0
        else:
            PM[base + r - 32, dp] = 1.0
    ONES = np.ones((128, 128), ml_dtypes.bfloat16)
    IDENT = np.eye(128, dtype=ml_dtypes.bfloat16)

    in_maps = []
    for core in range(8):
        b, g2 = core // 2, core % 2
        heads = list(range(g2 * NH, (g2 + 1) * NH))
        kvs = list(range(g2 * NKV, (g2 + 1) * NKV))

        xb = x[b]
        xhv = xb.astype(ml_dtypes.bfloat16)
        xlo = (xb - xhv.astype(np.float32)).astype(ml_dtypes.bfloat16)

        wq_c = Wq[:, g2 * NH * 128:(g2 + 1) * NH * 128]
        wk_c = Wk[:, g2 * NKV * 128:(g2 + 1) * NKV * 128]
        wv_c = Wv[:, g2 * NKV * 128:(g2 + 1) * NKV * 128]
        wo_c = Wo[g2 * NH * 128:(g2 + 1) * NH * 128, :]

        wq_p = wq_c.reshape(16, 128, NH * 128).transpose(1, 0, 2).copy()
        wk_p = wk_c.reshape(16, 128, NKV * 128).transpose(1, 0, 2).copy()
        wv_p = wv_c.reshape(16, 128, NKV * 128).transpose(1, 0, 2).copy()
        wo_p = wo_c.reshape(NH, 128, D).transpose(1, 0, 2).astype(ml_dtypes.bfloat16)

        bf = ml_dtypes.bfloat16
        cosq_p = np.ascontiguousarray(
            cos[:, heads, :][:, :, p64].transpose(2, 1, 0)).astype(bf)
        sinq_p = np.ascontiguousarray(
            sin[:, heads, :][:, :, p64].transpose(2, 1, 0)).astype(bf)
        cosk_p = np.ascontiguousarray(
            cos[:, kvs, :][:, :, p64].transpose(2, 1, 0)).astype(bf)
        sink_p = np.ascontiguousarray(
            sin[:, kvs, :][:, :, p64].transpose(2, 1, 0)).astype(bf)

        mb = np.where(mask[b].reshape(NB, 128).T.astype(bool), 0.0, -1e9)
        mb = mb.astype(np.float32)

        cv = np.ones((128, NH), np.float32)
        cv[0:64, :] = (-2.0 * np.tanh(qp[heads]))[None, :]
        alp = np.tile((np.exp(ls[heads]) / HD)[None, :], (128, 1))

        in_maps.append({
            "xh": xhv, "xl": xlo,
            "wq": wq_p.astype(np.float32), "wk": wk_p.astype(np.float32),
            "wv": wv_p.astype(np.float32), "wo": wo_p,
            "cosq": cosq_p, "sinq": sinq_p, "cosk": cosk_p, "sink": sink_p,
            "maskb": mb, "cvec": cv,
            "alpha": alp.astype(np.float32),
            "pmrot": PM.astype(bf), "onesb": ONES, "identb": IDENT,
        })
    return in_maps


def kernel(**inputs):
    if "nc" not in _CACHED:
        _CACHED["nc"] = build_program()
    nc = _CACHED["nc"]
    in_maps = _host_prep(**inputs)
    res = run_bass_kernel_spmd(nc, in_maps, list(range(8))).results
    out = np.empty((B, L, D), np.float32)
    for b in range(B):
        out[b] = (res[2 * b]["y"].astype(np.float32)
                  + res[2 * b + 1]["y"].astype(np.float32))
    return out


# revision 36
# speedup vs baseline: 1.0072x; 1.0050x over previous
"""BivectorRotarySelfAttention TRN2 kernel.

Sharding: 8 cores = 4 batches x 2 head-halves. Each core computes one batch's
attention for 8 heads (2 kv heads) and a partial output projection; host sums
the two head-half partials per batch.

v2 dataflow (features in partitions, seq in free):
  xT     = recombine(dma_transpose(x_hi), dma_transpose(x_lo))    [f32r]
  k/v/q  = W-blocks.T @ xT   (PSUM-accumulated f32r matmuls)
  rope   = PE permutation-matmul + 2 DVE muls + 1 add -> bf16
  kswap  = partition-swap of rope'd k via 2 SBUF->SBUF DMAs
  scores: per 256-col chunk one psum tile [S0|C0|S1|C1] (4 bf16 K=64 matmuls)
          Act copies [S1|C1] -> SBUF bf16; DVE: tp = [S0|C0]*[S1|C1];
          stt raw = tp_lo + c'*tp_hi written straight into the E tile (bf16)
  exp    in-place per stripe on Act (scale=alpha, bias=key-mask), causal
         triangle via GPSIMD affine_select on the diagonal block
  attnv  + ones-rowsum matmuls per 512-col half; DVE rcp + normalize -> bf16
  y[l,:] = sum_h outT_h.T @ Wo_h  (bf16 matmuls, f32 out)
  Next head's q-projection matmuls are interleaved into the scores phase to
  keep PE busy while DVE/Act post-process score chunks.
"""
import sys
if '/opt/trn_rl_repo' not in sys.path:
    sys.path.insert(0, '/opt/trn_rl_repo')

import numpy as np
import ml_dtypes

import concourse.bass as bass
import concourse.mybir as mybir
import concourse.tile as tile
from concourse import bacc
from concourse.bass_utils import run_bass_kernel_spmd

F32 = mybir.dt.float32
F32R = mybir.dt.float32r
BF16 = mybir.dt.bfloat16

B, L, D, H, HKV = 4, 1024, 2048, 16, 4
HD = D // H            # 128
HD2 = HD // 2          # 64
NH = 8                 # heads per core
NKV = 2                # kv heads per core
NB = L // 128          # 8 key blocks
AluOp = mybir.AluOpType
Act = mybir.ActivationFunctionType

_CACHED = {}


def _chunks_for_stripe(mb):
    """Q-column chunks [(qs, qe)] covering [128*mb, 1024) in <=256-col pieces."""
    q0 = 128 * mb
    out = []
    while q0 < L:
        qe = min(L, q0 + 256)
        out.append((q0, qe))
        q0 = qe
    return out


def build_program():
    nc = bacc.Bacc("TRN2", target_bir_lowering=False, debug=False)

    xh = nc.declare_dram_parameter("xh", [L, D], BF16, isOutput=False)
    xl = nc.declare_dram_parameter("xl", [L, D], BF16, isOutput=False)
    wq = nc.declare_dram_parameter("wq", [128, 16, NH * 128], F32R, isOutput=False)
    wk = nc.declare_dram_parameter("wk", [128, 16, NKV * 128], F32R, isOutput=False)
    wv = nc.declare_dram_parameter("wv", [128, 16, NKV * 128], F32R, isOutput=False)
    wo = nc.declare_dram_parameter("wo", [128, NH, D], BF16, isOutput=False)
    cosq = nc.declare_dram_parameter("cosq", [128, NH, L], BF16, isOutput=False)
    sinq = nc.declare_dram_parameter("sinq", [128, NH, L], BF16, isOutput=False)
    cosk = nc.declare_dram_parameter("cosk", [128, NKV, L], BF16, isOutput=False)
    sink = nc.declare_dram_parameter("sink", [128, NKV, L], BF16, isOutput=False)
    maskb = nc.declare_dram_parameter("maskb", [128, NB], F32, isOutput=False)
    cvec = nc.declare_dram_parameter("cvec", [128, NH], F32, isOutput=False)
    alpha = nc.declare_dram_parameter("alpha", [128, NH], F32, isOutput=False)
    pmrot = nc.declare_dram_parameter("pmrot", [128, 128], BF16, isOutput=False)
    onesb = nc.declare_dram_parameter("onesb", [128, 128], BF16, isOutput=False)
    identb = nc.declare_dram_parameter("identb", [128, 128], BF16, isOutput=False)
    y = nc.declare_dram_parameter("y", [L, D], BF16, isOutput=True)

    with tile.TileContext(nc) as tc:
        with tc.tile_pool(name="persist", bufs=1) as pp:
            consts = {}
            for nm, src, dt_ in [("pmrot", pmrot, BF16), ("onesb", onesb, BF16),
                                 ("identb", identb, BF16), ("maskb", maskb, F32),
                                 ("cvec", cvec, F32), ("alpha", alpha, F32)]:
                consts[nm] = pp.tile(list(src.shape), dt_, tag=nm, name=nm)

            def load_consts():
                for nm, src in [("pmrot", pmrot), ("onesb", onesb),
                                ("identb", identb), ("maskb", maskb),
                                ("cvec", cvec), ("alpha", alpha)]:
                    nc.sync.dma_start(consts[nm][:], src[:])

            # head-0 weights/tables live in the persist pool so their DMAs can
            # be sequenced inside the prologue's serial DMA pipe
            wq_h0 = pp.tile([128, 16, 128], F32R, tag="wq0", name="wq0")
            cq0 = pp.tile([128, L], BF16, tag="cq0", name="cq0")
            sq0 = pp.tile([128, L], BF16, tag="sq0", name="sq0")

            xt = [pp.tile([128, L], F32R, tag=f"xt{i}", name=f"xt{i}")
                  for i in range(16)]
            krt = [pp.tile([128, L], BF16, tag=f"krt{g}", name=f"krt{g}")
                   for g in range(NKV)]
            ksw = [pp.tile([128, L], BF16, tag=f"ksw{g}", name=f"ksw{g}")
                   for g in range(NKV)]
            vblk = [pp.tile([128, 128], BF16, tag=f"vb{i}", name=f"vb{i}")
                    for i in range(NKV * NB)]
            outtn = [pp.tile([128, L], BF16, tag=f"ot{h}", name=f"ot{h}")
                     for h in range(NH)]
            wo_t = [pp.tile([128, D], BF16, tag=f"wo{h}", name=f"wo{h}")
                    for h in range(NH)]

            # ---------------- prologue: xT, k proj + rope + swap, v proj + T
            with (
                tc.tile_pool(name="pro", bufs=1) as ppro,
                tc.tile_pool(name="psp", bufs=1, space="PSUM") as psp,
            ):
                # DMA issue order IS priority (one serial pipe): wk first so
                # kproj can start, x transposes next, wv before the v stream
                # needs it, consts/tables only before their first use
                wk_t = ppro.tile([128, 16, NKV * 128], F32R, tag="wk", name="wk_t")
                wv_t = ppro.tile([128, 16, NKV * 128], F32R, tag="wv", name="wv_t")

                def load_xt(ib):
                    th = ppro.tile([128, L], BF16, tag="xh_t", bufs=3, name="th")
                    tl = ppro.tile([128, L], BF16, tag="xl_t", bufs=3, name="tl")
                    nc.sync.dma_start_transpose(th[:], xh[:, ib * 128:(ib + 1) * 128])
                    nc.sync.dma_start_transpose(tl[:], xl[:, ib * 128:(ib + 1) * 128])
                    nc.vector.tensor_add(xt[ib][:], th[:], tl[:])

                nc.sync.dma_start(wk_t[:], wk[:])
                nc.sync.dma_start(wq_h0[:], wq[:, :, 0:128])
                load_xt(0)
                load_xt(1)
                nc.sync.dma_start(wv_t[:], wv[:])

                psk = [psp.tile([128, L], F32, tag=f"pjk{g}", name=f"psk{g}")
                       for g in range(NKV)]
                psv0 = psp.tile([128, L], F32, tag="pjv0", name="psv0")
                psq0 = psp.tile([128, L], F32, tag="pjq", name="psq0")

                def kv_mms(ps_list, w_t, ib):
                    for g in range(NKV):
                        for c in range(2):
                            nc.tensor.matmul(
                                ps_list[g][:, c * 512:(c + 1) * 512],
                                w_t[:, ib, g * 128:(g + 1) * 128],
                                xt[ib][:, c * 512:(c + 1) * 512],
                                start=(ib == 0), stop=(ib == 15))

                tabs = []

                def load_ktabs():
                    for g in range(NKV):
                        csl = ppro.tile([128, L], BF16, tag="ktab", bufs=4,
                                        name=f"csl{g}")
                        snl = ppro.tile([128, L], BF16, tag="ktab", bufs=4,
                                        name=f"snl{g}")
                        nc.sync.dma_start(csl[:], cosk[:, g, :])
                        nc.sync.dma_start(snl[:], sink[:, g, :])
                        tabs.append((csl, snl))

                # three streams interleaved per ib (k both kv-heads, q head 0,
                # v kv-head 0), paced by the x transposes; v kv-head 1 runs
                # after, overlapped with the k-rope and q-rope chains
                for ib in range(16):
                    if ib + 2 <= 15:
                        load_xt(ib + 2)
                    if ib == 8:
                        load_ktabs()
                    if ib == 11:
                        load_consts()
                        nc.sync.dma_start(cq0[:], cosq[:, 0, :])
                        nc.sync.dma_start(sq0[:], sinq[:, 0, :])
                    kv_mms(psk, wk_t, ib)
                    for c in range(2):
                        nc.tensor.matmul(
                            psq0[:, c * 512:(c + 1) * 512],
                            wq_h0[:, ib, :],
                            xt[ib][:, c * 512:(c + 1) * 512],
                            start=(ib == 0), stop=(ib == 15))
                    if ib >= 4:
                        for c in range(2):
                            nc.tensor.matmul(
                                psv0[:, c * 512:(c + 1) * 512],
                                wv_t[:, ib - 4, 0:128],
                                xt[ib - 4][:, c * 512:(c + 1) * 512],
                                start=(ib == 4), stop=(ib == 15))
                for ib in range(12, 16):
                    for c in range(2):
                        nc.tensor.matmul(
                            psv0[:, c * 512:(c + 1) * 512],
                            wv_t[:, ib, 0:128],
                            xt[ib][:, c * 512:(c + 1) * 512],
                            start=False, stop=(ib == 15))

                # head-0 q lands first so its rope chain overlaps the v g=1
                # projection below
                qt0 = pp.tile([128, L], BF16, tag="qt0p", name="qt0")
                qrt0 = pp.tile([128, L], BF16, tag="qrt0p", name="qrt0")
                qc0 = pp.tile([64, L], BF16, tag="qc0p", name="qc0")
                nc.scalar.copy(qt0[:], psq0[:])

                psv1 = psp.tile([128, L], F32, tag="pjq", name="psv1")
                for ib in range(16):
                    for c in range(2):
                        nc.tensor.matmul(
                            psv1[:, c * 512:(c + 1) * 512],
                            wv_t[:, ib, 128:256],
                            xt[ib][:, c * 512:(c + 1) * 512],
                            start=(ib == 0), stop=(ib == 15))

                for g in range(NKV):
                    kt = ppro.tile([128, L], BF16, tag="kt", bufs=2, name="kt")
                    nc.scalar.copy(kt[:], psk[g][:])
                    psr = psp.tile([128, L], F32, tag=f"pjk{g}", name="pskr")
                    for c in range(2):
                        nc.tensor.matmul(psr[:, c * 512:(c + 1) * 512],
                                         consts["pmrot"][:],
                                         kt[:, c * 512:(c + 1) * 512])
                    csl, snl = tabs[g]
                    t1 = ppro.tile([128, L], BF16, tag="rt", bufs=3, name="t1k")
                    t2 = ppro.tile([128, L], BF16, tag="rt", bufs=3, name="t2k")
                    nc.vector.tensor_mul(t1[:], psr[:], snl[:])
                    nc.vector.tensor_mul(t2[:], kt[:], csl[:])
                    nc.vector.tensor_add(krt[g][:], t1[:], t2[:])
                    # partition-swap halves via SBUF->SBUF DMA
                    nc.sync.dma_start(ksw[g][0:64, :], krt[g][64:128, :])
                    nc.sync.dma_start(ksw[g][64:128, :], krt[g][0:64, :])

                # head-0 rope (reuses the g=0 rope psum slot)
                psr0 = psp.tile([128, L], F32, tag="pjk0", name="psr0")
                for c in range(2):
                    nc.tensor.matmul(psr0[:, c * 512:(c + 1) * 512],
                                     consts["pmrot"][:],
                                     qt0[:, c * 512:(c + 1) * 512])
                t1q = ppro.tile([128, L], BF16, tag="rt", bufs=3, name="t1q")
                t2q = ppro.tile([128, L], BF16, tag="rt", bufs=3, name="t2q")
                nc.vector.tensor_mul(t1q[:], psr0[:], sq0[:])
                nc.vector.tensor_mul(t2q[:], qt0[:], cq0[:])
                nc.vector.tensor_add(qrt0[:], t1q[:], t2q[:])
                nc.vector.tensor_scalar_mul(qc0[:], qrt0[0:64, :],
                                            consts["cvec"][0:64, 0:1])

                for g, psv_g in ((0, psv0), (1, psv1)):
                    vt = ppro.tile([128, L], BF16, tag="vt", bufs=1, name="vt")
                    nc.scalar.copy(vt[:], psv_g[:])
                    # transpose blocks land in bf16 windows of the freed
                    # projection psum tile (no extra PSUM banks)
                    pv = psp.tile([128, L], F32, tag=f"pjk{g}", name="pv")
                    pvb = pv[:].bitcast(BF16)
                    for mb in range(NB):
                        nc.tensor.transpose(pvb[:, mb * 128:(mb + 1) * 128],
                                            vt[:, mb * 128:(mb + 1) * 128],
                                            consts["identb"][:])
                        nc.vector.tensor_copy(vblk[g * NB + mb][:],
                                              pvb[:, mb * 128:(mb + 1) * 128])

            # ---------------- head loop
            with (
                tc.tile_pool(name="hl", bufs=1) as ph,
                tc.tile_pool(name="psh", bufs=1, space="PSUM") as psh,
            ):
                def emit_qproj_dma(h):
                    wq_h = ph.tile([128, 16, 128], F32R, tag="wq_h", bufs=2,
                                   name=f"wq{h}")
                    nc.sync.dma_start(wq_h[:], wq[:, :, h * 128:(h + 1) * 128])
                    cq = ph.tile([128, L], BF16, tag="cq", bufs=2, name=f"cq{h}")
                    sq = ph.tile([128, L], BF16, tag="sq", bufs=2, name=f"sq{h}")
                    nc.sync.dma_start(cq[:], cosq[:, h, :])
                    nc.sync.dma_start(sq[:], sinq[:, h, :])
                    return wq_h, cq, sq

                def qproj_mm_thunks(wq_h, psq):
                    out = []
                    for ib in range(16):
                        for c in range(2):
                            def mm(ib=ib, c=c):
                                nc.tensor.matmul(
                                    psq[:, c * 512:(c + 1) * 512],
                                    wq_h[:, ib, :],
                                    xt[ib][:, c * 512:(c + 1) * 512],
                                    start=(ib == 0), stop=(ib == 15))
                            out.append(mm)
                    return out

                def emit_qtcopy(h, psq):
                    qt = ph.tile([128, L], BF16, tag="qt", bufs=2, name=f"qt{h}")
                    nc.scalar.copy(qt[:], psq[:])
                    return qt

                def emit_rope(h, qt, cq, sq):
                    psr = psh.tile([128, L], F32, tag="pjh", bufs=1,
                                   name=f"psr{h}")
                    for c in range(2):
                        nc.tensor.matmul(psr[:, c * 512:(c + 1) * 512],
                                         consts["pmrot"][:],
                                         qt[:, c * 512:(c + 1) * 512])
                    t1 = ph.tile([128, L], BF16, tag="t1", bufs=1, name="t1")
                    t2 = ph.tile([128, L], BF16, tag="t2", bufs=1, name="t2")
                    qrt = ph.tile([128, L], BF16, tag="qrt", bufs=2,
                                  name=f"qrt{h}")
                    qc = ph.tile([64, L], BF16, tag="qc", bufs=1, name=f"qc{h}")
                    nc.vector.tensor_mul(t1[:], psr[:], sq[:])
                    nc.vector.tensor_mul(t2[:], qt[:], cq[:])
                    nc.vector.tensor_add(qrt[:], t1[:], t2[:])
                    # C0-side q half pre-scaled by c' (folds the bivector
                    # coefficient into the matmul so the chunk combine is a
                    # plain bf16 add)
                    nc.vector.tensor_scalar_mul(qc[:], qrt[0:64, :],
                                                consts["cvec"][0:64, h:h + 1])
                    return qrt, qc

                def attnv_thunks(h, et):
                    """PE matmuls + rcp/normalize for head h's attention*V,
                    as thunks for interleaving into the next head's scores."""
                    g = h // 4
                    out = []
                    for c in range(2):
                        mbs = [mb for mb in range(NB) if 128 * mb < 512 * (c + 1)]

                        def open_group(c=c):
                            po = psh.tile([128, 512], F32, tag="po", bufs=1,
                                          name="po")
                            prs = psh.tile([128, 512], F32, tag="prs", bufs=1,
                                           name="prs")
                            return po, prs
                        state = {}
                        for i, mb in enumerate(mbs):
                            os_ = max(512 * c, 128 * mb)
                            oe = 512 * (c + 1)
                            st, sp = (i == 0), (i == len(mbs) - 1)

                            def mm(c=c, mb=mb, os_=os_, oe=oe, st=st, sp=sp,
                                   state=state, open_group=open_group):
                                if st:
                                    state["po"], state["prs"] = open_group()
                                esl = et[mb][:, os_ - 128 * mb: oe - 128 * mb]
                                nc.tensor.matmul(
                                    state["po"][:, os_ - 512 * c: oe - 512 * c],
                                    vblk[g * NB + mb][:], esl,
                                    start=st, stop=sp)
                                nc.tensor.matmul(
                                    state["prs"][:, os_ - 512 * c: oe - 512 * c],
                                    consts["onesb"][:], esl,
                                    start=st, stop=sp)
                            out.append(mm)

                        def post(h=h, c=c, state=state):
                            rcp = ph.tile([128, 512], F32, tag="rcp", bufs=1,
                                          name="rcp")
                            nc.vector.reciprocal_approx_fast(rcp[:],
                                                             state["prs"][:])
                            nc.gpsimd.tensor_mul(
                                outtn[h][:, 512 * c:512 * (c + 1)],
                                state["po"][:], rcp[:])
                        out.append(post)
                    return out

                # head 0's projection + rope were computed in the prologue
                qrt_cur, qc_cur = qrt0, qc0
                av_prev = []

                for h in range(NH):
                    g = h // 4
                    if h + 1 < NH:
                        wq_hn, cqn, sqn = emit_qproj_dma(h + 1)
                        psqn = psh.tile([128, L], F32, tag="pjh", bufs=1,
                                        name=f"psq{h+1}")
                        qp_thunks = qproj_mm_thunks(wq_hn, psqn)
                    else:
                        qp_thunks = []
                    qp_i = 0
                    av_i = 0
                    qt_next = None
                    rope_next = None

                    et = []
                    for mb in range(NB):
                        w = L - 128 * mb
                        et.append(ph.tile([128, w], BF16, tag=f"et{mb}", bufs=2,
                                          name=f"et{h}_{mb}"))

                    chunk_idx = 0
                    for mb in range(NB):
                        kb = slice(mb * 128, (mb + 1) * 128)
                        for (qs, qe) in _chunks_for_stripe(mb):
                            s = qe - qs
                            ps = psh.tile([128, 1024], F32, tag="sc", bufs=2,
                                          name="sc")
                            nc.tensor.matmul(ps[:, 2 * s:3 * s],
                                             krt[g][64:128, kb],
                                             qrt_cur[64:128, qs:qe])
                            nc.tensor.matmul(ps[:, 3 * s:4 * s],
                                             ksw[g][64:128, kb],
                                             qrt_cur[64:128, qs:qe])
                            nc.tensor.matmul(ps[:, 0:s],
                                             krt[g][0:64, kb],
                                             qrt_cur[0:64, qs:qe])
                            nc.tensor.matmul(ps[:, s:2 * s],
                                             ksw[g][0:64, kb],
                                             qc_cur[0:64, qs:qe])
                            bsB = ph.tile([128, 512], BF16, tag="bsB", bufs=3,
                                          name="bsB")
                            nc.scalar.copy(bsB[:, 0:2 * s], ps[:, 2 * s:4 * s])
                            tp = ph.tile([128, 512], BF16, tag="tp", bufs=3,
                                         name="tp")
                            nc.vector.tensor_mul(tp[:, 0:2 * s],
                                                 ps[:, 0:2 * s],
                                                 bsB[:, 0:2 * s])
                            esl = et[mb][:, qs - 128 * mb: qe - 128 * mb]
                            nc.vector.tensor_add(esl, tp[:, 0:s],
                                                 tp[:, s:2 * s])
                            # interleave filler PE work (next head's q
                            # projection + previous head's attnV),
                            # proportionally paced across the 20 chunks
                            qp_tgt = (chunk_idx + 1) * len(qp_thunks) // 18
                            while qp_i < min(qp_tgt, len(qp_thunks)):
                                qp_thunks[qp_i]()
                                qp_i += 1
                            av_tgt = (chunk_idx + 1) * len(av_prev) // 19
                            while av_i < min(av_tgt, len(av_prev)):
                                av_prev[av_i]()
                                av_i += 1
                            if qp_thunks and qp_i >= len(qp_thunks) \
                                    and qt_next is None:
                                qt_next = emit_qtcopy(h + 1, psqn)
                            elif qt_next is not None and rope_next is None:
                                rope_next = emit_rope(h + 1, qt_next, cqn, sqn)
                            chunk_idx += 1
                        # stripe done: exp in place, causal triangle on diagonal
                        nc.scalar.activation(et[mb][:], et[mb][:], Act.Exp,
                                             bias=consts["maskb"][:, mb:mb + 1],
                                             scale=consts["alpha"][:, h:h + 1])
                        nc.gpsimd.affine_select(
                            et[mb][:, 0:128], et[mb][:, 0:128],
                            pattern=[[1, 128]], compare_op=AluOp.is_ge,
                            fill=0.0, base=0, channel_multiplier=-1)

                    while qp_i < len(qp_thunks):
                        qp_thunks[qp_i]()
                        qp_i += 1
                    while av_i < len(av_prev):
                        av_prev[av_i]()
                        av_i += 1
                    if h + 1 < NH and qt_next is None:
                        qt_next = emit_qtcopy(h + 1, psqn)
                    if 3 <= h <= 6:
                        for hb in (2 * (h - 3), 2 * (h - 3) + 1):
                            nc.sync.dma_start(wo_t[hb][:], wo[:, hb, :])

                    av_prev = attnv_thunks(h, et)
                    if h + 1 < NH:
                        if rope_next is None:
                            rope_next = emit_rope(h + 1, qt_next, cqn, sqn)
                        qrt_cur, qc_cur = rope_next
                    else:
                        for t in av_prev:
                            t()
                        av_prev = []

            # ---------------- epilogue: Wo projection
            with (
                tc.tile_pool(name="ep", bufs=1) as pe2,
                tc.tile_pool(name="pse", bufs=1, space="PSUM") as pse,
            ):
                for lb in range(NB):
                    for c in range(2):
                        psy = pse.tile([128, 1024], F32, tag="py", bufs=2,
                                       name="psy")
                        for cc in range(2):
                            for hh in range(NH):
                                nc.tensor.matmul(
                                    psy[:, cc * 512:(cc + 1) * 512],
                                    outtn[hh][:, lb * 128:(lb + 1) * 128],
                                    wo_t[hh][:, c * 1024 + cc * 512:
                                              c * 1024 + (cc + 1) * 512],
                                    start=(hh == 0), stop=(hh == NH - 1))
                        yt = pe2.tile([128, 1024], BF16, tag="yt", bufs=4,
                                      name="yt")
                        if (lb * 2 + c) % 2 == 0:
                            nc.scalar.copy(yt[:], psy[:])
                        else:
                            nc.vector.tensor_copy(yt[:], psy[:])
                        nc.sync.dma_start(
                            y[lb * 128:(lb + 1) * 128,
                              c * 1024:(c + 1) * 1024], yt[:])

    nc.compile()
    return nc


def _host_prep(x, Wq, Wk, Wv, Wo, q_param, log_scale, cos, sin, mask):
    """Build the 8 per-core input maps."""
    x = np.asarray(x, np.float32)
    Wq = np.asarray(Wq, np.float32)
    Wk = np.asarray(Wk, np.float32)
    Wv = np.asarray(Wv, np.float32)
    Wo = np.asarray(Wo, np.float32)
    cos = np.asarray(cos, np.float32)[0]      # [L, H, 64]
    sin = np.asarray(sin, np.float32)[0]
    qp = np.asarray(q_param, np.float32).reshape(H)
    ls = np.asarray(log_scale, np.float32).reshape(H)
    mask = np.asarray(mask)

    p64 = np.arange(128) % 64

    PM = np.zeros((128, 128), np.float32)
    for dp in range(128):
        base, r = (dp // 64) * 64, dp % 64
        if r < 32:
            PM[base + r + 32, dp] = -1.0
        else:
            PM[base + r - 32, dp] = 1.0
    ONES = np.ones((128, 128), ml_dtypes.bfloat16)
    IDENT = np.eye(128, dtype=ml_dtypes.bfloat16)

    in_maps = []
    for core in range(8):
        b, g2 = core // 2, core % 2
        heads = list(range(g2 * NH, (g2 + 1) * NH))
        kvs = list(range(g2 * NKV, (g2 + 1) * NKV))

        xb = x[b]
        xhv = xb.astype(ml_dtypes.bfloat16)
        xlo = (xb - xhv.astype(np.float32)).astype(ml_dtypes.bfloat16)

        wq_c = Wq[:, g2 * NH * 128:(g2 + 1) * NH * 128]
        wk_c = Wk[:, g2 * NKV * 128:(g2 + 1) * NKV * 128]
        wv_c = Wv[:, g2 * NKV * 128:(g2 + 1) * NKV * 128]
        wo_c = Wo[g2 * NH * 128:(g2 + 1) * NH * 128, :]

        wq_p = wq_c.reshape(16, 128, NH * 128).transpose(1, 0, 2).copy()
        wk_p = wk_c.reshape(16, 128, NKV * 128).transpose(1, 0, 2).copy()
        wv_p = wv_c.reshape(16, 128, NKV * 128).transpose(1, 0, 2).copy()
        wo_p = wo_c.reshape(NH, 128, D).transpose(1, 0, 2).astype(ml_dtypes.bfloat16)

        bf = ml_dtypes.bfloat16
        cosq_p = np.ascontiguousarray(
            cos[:, heads, :][:, :, p64].transpose(2, 1, 0)).astype(bf)
        sinq_p = np.ascontiguousarray(
            sin[:, heads, :][:, :, p64].transpose(2, 1, 0)).astype(bf)
        cosk_p = np.ascontiguousarray(
            cos[:, kvs, :][:, :, p64].transpose(2, 1, 0)).astype(bf)
        sink_p = np.ascontiguousarray(
            sin[:, kvs, :][:, :, p64].transpose(2, 1, 0)).astype(bf)

        mb = np.where(mask[b].reshape(NB, 128).T.astype(bool), 0.0, -1e9)
        mb = mb.astype(np.float32)

        cv = np.ones((128, NH), np.float32)
        cv[0:64, :] = (-2.0 * np.tanh(qp[heads]))[None, :]
        alp = np.tile((np.exp(ls[heads]) / HD)[None, :], (128, 1))

        in_maps.append({
            "xh": xhv, "xl": xlo,
            "wq": wq_p.astype(np.float32), "wk": wk_p.astype(np.float32),
            "wv": wv_p.astype(np.float32), "wo": wo_p,
            "cosq": cosq_p, "sinq": sinq_p, "cosk": cosk_p, "sink": sink_p,
            "maskb": mb, "cvec": cv,
            "alpha": alp.astype(np.float32),
            "pmrot": PM.astype(bf), "onesb": ONES, "identb": IDENT,
        })
    return in_maps


def kernel(**inputs):
    if "nc" not in _CACHED:
        _CACHED["nc"] = build_program()
    nc = _CACHED["nc"]
    in_maps = _host_prep(**inputs)
    res = run_bass_kernel_spmd(nc, in_maps, list(range(8))).results
    out = np.empty((B, L, D), np.float32)
    for b in range(B):
        out[b] = (res[2 * b]["y"].astype(np.float32)
                  + res[2 * b + 1]["y"].astype(np.float32))
    return out
